# revision 9
# baseline (speedup 1.0000x reference)
"""
DynamicSparseAttention Trainium2 Bass kernel.

Reference computation (per batch b, head h):
    scores = (Q @ K^T) / sqrt(64)                      # [S, S]
    var    = row-variance of scores (ddof=1)           # [S, 1]
    tau    = max(1/(1+var), 0.3)
    attn   = softmax(scores / tau, axis=-1)
    out    = attn @ V
    returns (out, attn, tau)

Sharding: B*H = 64 heads, split 8 heads per NeuronCore (8 cores).

Per-core algorithm (all fp32):
  * Row variance of scores is computed WITHOUT touching the S x S matrix,
    via Gram matrices:  sum_j s_ij   = (q_i . ksum) / 8
                        sum_j s_ij^2 = q_i^T (K^T K) q_i / 64
    so tau (and qscale = 1/(8*tau)) are known before the big matmuls.
  * Orientation-2 pass: s_T[k, q] = K @ Q'^T (Q' = Q * qscale rows) ->
    exp on ScalarE -> expT in SBUF.  attn@[V|1] via accumulating matmuls
    with V as the stationary operand gives unnormalized out^T and the
    softmax denominator Z in one shot.
  * Orientation-1 pass: s[q, k] = Q' @ K^T -> exp -> multiply by
    r = 1/Z (per-partition scalar on VectorE) -> attn rows -> DMA to HBM.
  * out = out_unnorm^T transposed (PE) and scaled by r.
"""

import math
import numpy as np
from contextlib import ExitStack

B, H, S, D = 4, 16, 2048, 64
N_CORES = 8
HPC = (B * H) // N_CORES      # heads per core
P = 128                       # partitions
NT = S // P                   # q/k sub-blocks of 128 rows

_CACHE = {}


def _build_program(hpc, s, d, n_reps=1):
    import concourse.bass as bass
    import concourse.tile as tile
    from concourse import bacc, mybir

    f32 = mybir.dt.float32
    f32r = mybir.dt.float32r
    f16 = mybir.dt.float16
    Exp = mybir.ActivationFunctionType.Exp
    Copy = mybir.ActivationFunctionType.Copy
    nt = s // P
    gq = min(8, nt)           # q sub-blocks per group
    ngroups = nt // gq
    gw = gq * P               # group width in q (<= 1024)
    kk = min(8, nt)           # k-chunks per transpose batch
    nkb = nt // kk

    nc = bacc.Bacc(
        "TRN2",
        target_bir_lowering=False,
        debug=False,
        enable_asserts=False,
        num_devices=N_CORES,
    )

    q_d = nc.dram_tensor("q", [hpc, s, d], f32, kind="ExternalInput")
    k_d = nc.dram_tensor("k", [hpc, s, d], f32, kind="ExternalInput")
    v_d = nc.dram_tensor("v", [hpc, s, d], f32, kind="ExternalInput")
    attn_d = nc.dram_tensor("attn", [hpc, s, s], f32, kind="ExternalOutput")
    out_d = nc.dram_tensor("out", [hpc, s, d], f32, kind="ExternalOutput")
    tau_d = nc.dram_tensor("tau", [hpc, s, 1], f32, kind="ExternalOutput")

    ident_dram = nc.inline_tensor(np.eye(P, dtype=np.float32), name="ident")

    c1 = 1.0 / (float(d) * float(s - 1))              # ss2 -> E[s^2] term
    c2 = 1.0 / (float(d) * float(s) * float(s - 1))   # ss1^2 term

    with tile.TileContext(nc) as tc, ExitStack() as ctx:
        const_pool = ctx.enter_context(tc.tile_pool(name="const", bufs=1))
        ld_pool = ctx.enter_context(tc.tile_pool(name="ld", bufs=2))
        der_pool = ctx.enter_context(tc.tile_pool(name="der", bufs=2))
        stats_pool = ctx.enter_context(tc.tile_pool(name="stats", bufs=2))
        sb_misc = ctx.enter_context(tc.tile_pool(name="sbmisc", bufs=2))
        expT_pool = ctx.enter_context(tc.tile_pool(name="expT", bufs=1))
        row_pool = ctx.enter_context(tc.tile_pool(name="row", bufs=3))
        mm_psum = ctx.enter_context(tc.tile_pool(name="mmps", bufs=2, space="PSUM"))
        av_psum = ctx.enter_context(tc.tile_pool(name="avps", bufs=1, space="PSUM"))
        misc_psum = ctx.enter_context(tc.tile_pool(name="mips", bufs=2, space="PSUM"))

        ident = const_pool.tile([P, P], f32)
        nc.sync.dma_start(ident[:], ident_dram.ap())
        ident16 = const_pool.tile([P, P], f16)
        nc.vector.tensor_copy(ident16[:], ident[:])
        ident_r = const_pool.tile([P, P], f32r)
        nc.vector.tensor_copy(ident_r[:], ident[:])

        for h in [hh for _ in range(n_reps) for hh in range(hpc)]:
            # ---------------- loads ----------------
            qt = ld_pool.tile([P, nt * d], f32, tag="qt")
            nc.sync.dma_start(
                qt[:].rearrange("p (t d) -> p t d", d=d),
                q_d.ap()[h].rearrange("(t p) d -> p t d", p=P),
            )
            kt = ld_pool.tile([P, nt * (d + 1)], f32, tag="kt")
            kt3 = kt[:].rearrange("p (t c) -> p t c", c=d + 1)
            nc.sync.dma_start(
                kt3[:, :, 0:d], k_d.ap()[h].rearrange("(t p) d -> p t d", p=P)
            )
            nc.vector.memset(kt3[:, :, d : d + 1], 1.0)
            vt = ld_pool.tile([P, nt * (d + 1)], f32, tag="vt")
            vt3 = vt[:].rearrange("p (t c) -> p t c", c=d + 1)
            nc.sync.dma_start(
                vt3[:, :, 0:d], v_d.ap()[h].rearrange("(t p) d -> p t d", p=P)
            )
            nc.vector.memset(vt3[:, :, d : d + 1], 1.0)
            vt_r = der_pool.tile([P, nt * (d + 1)], f32r, tag="vt_r")
            nc.vector.tensor_copy(vt_r[:], vt[:])
            vt_r3 = vt_r[:].rearrange("p (t c) -> p t c", c=d + 1)

            # ---------------- Gram: M_aug = [K^T K | ksum] ----------------
            ps_m = misc_psum.tile([d + 1, d], f32, tag="mips")
            for kc in range(nt):
                nc.tensor.matmul(
                    ps_m[:],
                    kt3[:, kc, 0 : d + 1],
                    kt3[:, kc, 0:d],
                    start=(kc == 0),
                    stop=(kc == nt - 1),
                )
            m_sb = sb_misc.tile([d + 1, d], f32, tag="m_sb")
            nc.vector.tensor_copy(m_sb[:], ps_m[:])
            ps_mt = misc_psum.tile([d, d + 1], f32, tag="mips")
            nc.tensor.transpose(ps_mt[:], m_sb[:], ident[0 : d + 1, 0 : d + 1])
            m_aug = sb_misc.tile([d, d + 1], f32, tag="m_aug")
            nc.vector.tensor_copy(m_aug[:], ps_mt[:])

            # ---------------- qT (unscaled Q transposed, fp32 for QM) ------
            gt = min(4, nt)
            qT = der_pool.tile([d, s], f32, tag="qT")
            for g in range(nt // gt):
                ps_t = misc_psum.tile([d, gt * P], f32, tag="mips")
                for j in range(gt):
                    t = g * gt + j
                    nc.tensor.transpose(
                        ps_t[:, j * P : (j + 1) * P],
                        qt[:, t * d : (t + 1) * d],
                        ident[:],
                    )
                nc.vector.tensor_copy(qT[:, g * gt * P : (g + 1) * gt * P], ps_t[:])

            # ---------------- QM + row stats -> tau, qscale ----------------
            ss1 = stats_pool.tile([P, nt], f32, tag="ss1")
            ss2 = stats_pool.tile([P, nt], f32, tag="ss2")
            for grp in range(ngroups):
                ps_qm = mm_psum.tile([P, gw], f32, tag="mm")
                for j in range(gq):
                    t = grp * gq + j
                    nc.tensor.matmul(
                        ps_qm[:, j * P : j * P + (d + 1)],
                        qT[:, t * P : (t + 1) * P],
                        m_aug[:],
                        start=True,
                        stop=True,
                    )
                ps_qm3 = ps_qm[:].rearrange("p (j c) -> p j c", c=P)
                prod = sb_misc.tile([P, gq * d], f32, tag="prod")
                nc.vector.tensor_mul(
                    prod[:].rearrange("p (j c) -> p j c", c=d),
                    ps_qm3[:, :, 0:d],
                    qt[:].rearrange("p (t c) -> p t c", c=d)[
                        :, grp * gq : (grp + 1) * gq, :
                    ],
                )
                nc.vector.reduce_sum(
                    ss2[:, grp * gq : (grp + 1) * gq],
                    prod[:].rearrange("p (j c) -> p j c", c=d),
                    axis=mybir.AxisListType.X,
                )
                nc.vector.tensor_copy(
                    ss1[:, grp * gq : (grp + 1) * gq], ps_qm3[:, :, d]
                )

            var = stats_pool.tile([P, nt], f32, tag="var")
            t1 = stats_pool.tile([P, nt], f32, tag="t1")
            nc.vector.tensor_mul(t1[:], ss1[:], ss1[:])
            nc.vector.tensor_scalar_mul(var[:], ss2[:], c1)
            nc.vector.tensor_scalar_mul(t1[:], t1[:], c2)
            nc.vector.tensor_sub(var[:], var[:], t1[:])
            tau_all = stats_pool.tile([P, nt], f32, tag="tau_all")
            nc.vector.tensor_scalar_add(var[:], var[:], 1.0)
            nc.vector.reciprocal(tau_all[:], var[:])
            nc.vector.tensor_scalar_max(tau_all[:], tau_all[:], 0.3)
            qscale = stats_pool.tile([P, nt], f32, tag="qscale")
            nc.vector.reciprocal(qscale[:], tau_all[:])
            nc.vector.tensor_scalar_mul(qscale[:], qscale[:], 1.0 / 8.0)

            ps_tt = misc_psum.tile([nt, P], f32, tag="mips")
            nc.tensor.transpose(ps_tt[:], tau_all[:], ident[:])
            tau_sb = sb_misc.tile([nt, P], f32, tag="tau_sb")
            nc.vector.tensor_copy(tau_sb[:], ps_tt[:])
            nc.sync.dma_start(
                tau_d.ap()[h].rearrange("(t p) one -> t (p one)", p=P), tau_sb[:]
            )

            # ------- Q' = Q * qscale; fp16 hi/lo splits of Q'^T and K^T -----
            q2 = der_pool.tile([P, nt * d], f32, tag="q2")
            for t in range(nt):
                nc.vector.tensor_scalar_mul(
                    q2[:, t * d : (t + 1) * d],
                    qt[:, t * d : (t + 1) * d],
                    qscale[:, t : t + 1],
                )
            q2Th = der_pool.tile([d, s], f16, tag="q2Th")
            q2Tl = der_pool.tile([d, s], f16, tag="q2Tl")
            kTh = der_pool.tile([d, s], f16, tag="kTh")
            kTl = der_pool.tile([d, s], f16, tag="kTl")
            for g in range(nt // gt):
                sl = slice(g * gt * P, (g + 1) * gt * P)
                ps_t = misc_psum.tile([d, gt * P], f32, tag="mips")
                for j in range(gt):
                    t = g * gt + j
                    nc.tensor.transpose(
                        ps_t[:, j * P : (j + 1) * P],
                        q2[:, t * d : (t + 1) * d],
                        ident[:],
                    )
                nc.vector.tensor_copy(q2Th[:, sl], ps_t[:])
                nc.vector.tensor_sub(q2Tl[:, sl], ps_t[:], q2Th[:, sl])
                ps_t2 = misc_psum.tile([d, gt * P], f32, tag="mips")
                for j in range(gt):
                    t = g * gt + j
                    nc.tensor.transpose(
                        ps_t2[:, j * P : (j + 1) * P],
                        kt3[:, t, 0:d],
                        ident[:],
                    )
                nc.vector.tensor_copy(kTh[:, sl], ps_t2[:])
                nc.vector.tensor_sub(kTl[:, sl], ps_t2[:], kTh[:, sl])

            # ---------------- main loop over q groups ----------------
            r_all = stats_pool.tile([P, nt], f32, tag="r_all")
            out_head = stats_pool.tile([P, nt * d], f32, tag="out_head")

            for grp in range(ngroups):
                q0 = grp * gw
                cks = [(c0, min(512, gw - c0)) for c0 in range(0, gw, 512)]
                av = av_psum.tile([d + 1, gw], f32, tag="av")
                expT = expT_pool.tile([P, nt * gw], f32r, tag="expT")
                for kc in range(nt):
                    ksl = slice(kc * P, (kc + 1) * P)
                    ps_s = mm_psum.tile([P, gw], f32, tag="mm")
                    for c0, w in cks:
                        qsl = slice(q0 + c0, q0 + c0 + w)
                        nc.tensor.matmul(
                            ps_s[:, c0 : c0 + w], kTh[:, ksl], q2Th[:, qsl],
                            start=True, stop=False,
                        )
                        nc.tensor.matmul(
                            ps_s[:, c0 : c0 + w], kTh[:, ksl], q2Tl[:, qsl],
                            start=False, stop=False,
                        )
                        nc.tensor.matmul(
                            ps_s[:, c0 : c0 + w], kTl[:, ksl], q2Th[:, qsl],
                            start=False, stop=True,
                        )
                    nc.scalar.activation(
                        expT[:, kc * gw : (kc + 1) * gw], ps_s[:], Exp
                    )
                    for c0, w in cks:
                        nc.tensor.matmul(
                            av[:, c0 : c0 + w],
                            vt_r3[:, kc, 0 : d + 1],
                            expT[:, kc * gw + c0 : kc * gw + c0 + w],
                            start=(kc == 0),
                            stop=(kc == nt - 1),
                        )

                # out^T -> transpose -> out rows; Z -> r
                av_sb = sb_misc.tile([d + 1, gw], f32, tag="av_sb")
                nc.vector.tensor_copy(av_sb[:], av[:])
                avT = sb_misc.tile([P, gq * (d + 1)], f32, tag="avT")
                avT3 = avT[:].rearrange("p (j c) -> p j c", c=d + 1)
                for g in range((gq + 3) // 4):
                    nj = min(4, gq - g * 4)
                    ps_avt = misc_psum.tile([P, 4 * P], f32, tag="mips")
                    for j in range(nj):
                        nc.tensor.transpose(
                            ps_avt[:, j * P : j * P + (d + 1)],
                            av_sb[:, (g * 4 + j) * P : (g * 4 + j + 1) * P],
                            ident[0 : d + 1, 0 : d + 1],
                        )
                    nc.vector.tensor_copy(
                        avT3[:, g * 4 : g * 4 + nj, :],
                        ps_avt[:].rearrange("p (j c) -> p j c", c=P)[
                            :, 0:nj, 0 : d + 1
                        ],
                    )
                nc.vector.reciprocal(
                    r_all[:, grp * gq : (grp + 1) * gq], avT3[:, :, d]
                )
                for j in range(gq):
                    t = grp * gq + j
                    nc.vector.tensor_scalar_mul(
                        out_head[:, t * d : (t + 1) * d],
                        avT3[:, j, 0:d],
                        r_all[:, t : t + 1],
                    )

                # attn rows: PE-transpose expT chunks, normalize in the copy
                for j in range(gq):
                    t = grp * gq + j
                    row = row_pool.tile([P, s], f32, tag="row")
                    for kb in range(nkb):
                        ps_T = mm_psum.tile([P, kk * P], f32r, tag="mm")
                        for k2 in range(kk):
                            kc = kb * kk + k2
                            nc.tensor.transpose(
                                ps_T[:, k2 * P : (k2 + 1) * P],
                                expT[:, kc * gw + j * P : kc * gw + (j + 1) * P],
                                ident_r[:],
                            )
                        if (j + kb) % 2 == 0:
                            nc.vector.tensor_scalar_mul(
                                row[:, kb * kk * P : (kb + 1) * kk * P],
                                ps_T[:],
                                r_all[:, t : t + 1],
                            )
                        else:
                            nc.scalar.activation(
                                row[:, kb * kk * P : (kb + 1) * kk * P],
                                ps_T[:],
                                Copy,
                                scale=r_all[:, t : t + 1],
                            )
                    nc.sync.dma_start(attn_d.ap()[h, t * P : (t + 1) * P, :], row[:])

            nc.sync.dma_start(
                out_d.ap()[h].rearrange("(t p) d -> p t d", p=P),
                out_head[:].rearrange("p (t d) -> p t d", d=d),
            )

    nc.compile()
    return nc


def get_program(hpc=HPC, s=S, d=D, n_reps=1):
    key = (hpc, s, d, n_reps)
    if key not in _CACHE:
        _CACHE[key] = _build_program(hpc, s, d, n_reps)
    return _CACHE[key]


def kernel(q: np.ndarray, k: np.ndarray, v: np.ndarray):
    from concourse.bass_utils import run_bass_kernel_spmd

    q = np.ascontiguousarray(np.asarray(q, dtype=np.float32))
    k = np.ascontiguousarray(np.asarray(k, dtype=np.float32))
    v = np.ascontiguousarray(np.asarray(v, dtype=np.float32))
    b, hh, s, dd = q.shape

    nc = get_program(HPC, s, dd)

    qf = q.reshape(b * hh, s, dd)
    kf = k.reshape(b * hh, s, dd)
    vf = v.reshape(b * hh, s, dd)
    in_maps = []
    for c in range(N_CORES):
        sl = slice(c * HPC, (c + 1) * HPC)
        in_maps.append({"q": qf[sl], "k": kf[sl], "v": vf[sl]})

    res = run_bass_kernel_spmd(nc, in_maps, core_ids=list(range(N_CORES)))

    out = np.concatenate([res.results[c]["out"] for c in range(N_CORES)], axis=0)
    attn = np.concatenate([res.results[c]["attn"] for c in range(N_CORES)], axis=0)
    tau = np.concatenate([res.results[c]["tau"] for c in range(N_CORES)], axis=0)

    out = out.reshape(b, hh, s, dd).astype(np.float32, copy=False)
    attn = attn.reshape(b, hh, s, s).astype(np.float32, copy=False)
    tau = tau.reshape(b, hh, s, 1).astype(np.float32, copy=False)
    return (out, attn, tau)
